# revision 11
# baseline (speedup 1.0000x reference)
"""Trainium2 Bass kernel for masked cross-attention (nn_Attention_21440476741938).

Reference computation (b=2, n=4096, n_txt=128, c=1536, c_ctx=4096, h=24, d=64):
    q = x @ Wq;  k = context @ Wk;  v = context @ Wv        (multi-head, d=64)
    out = softmax(q k^T / sqrt(d) + mask) v;  y = out @ Wo

Sharding across 8 NeuronCores: core i -> batch b=i//4, quarter j=i%4.
  Phase 1: core computes k/v projections for its 384 feature columns
           (6 heads) of its batch, then a 4-core AllGather shares full
           K^T / V per batch.
  Phase 2: core runs attention + output projection for its 1024 query
           tokens (all 24 heads).  Outputs are disjoint -> no collective.

Matmuls use float32r operand views (full-rate fp32 on TensorE at N>=256).
Host pre-transposes x / context so contraction dims land on partitions.
"""

import numpy as np

import concourse.bass as bass
import concourse.bacc as bacc
import concourse.mybir as mybir
import concourse.tile as tile
from concourse.bass_utils import run_bass_kernel_spmd

F32 = mybir.dt.float32
F32R = mybir.dt.float32r
BF16 = mybir.dt.bfloat16

B, NQ, NKV, CIN, CCTX, C = 2, 4096, 128, 1536, 4096, 1536
H, D = 24, 64
SCALE = float(D) ** -0.5
NCORES = 8
QTOK = NQ * B // NCORES          # 1024 query tokens per core
FSH = C // 4                     # 384 feature columns per core in phase 1
NCH = C // 128                   # 12 feature chunks
CCH = CCTX // 128                # 32 context-feature chunks
MASK_NEG = -60.0                 # exp(-60) ~ 8.8e-27: negligible vs valid terms


def _r(ap):
    return ap.bitcast(F32R)


def build_nc():
    nc = bacc.Bacc("TRN2", target_bir_lowering=False, debug=False,
                   num_devices=NCORES)

    xT = nc.dram_tensor("xT", [CIN, QTOK], F32, kind="ExternalInput").ap()
    ctxT = nc.dram_tensor("ctxT", [CCTX, NKV], F32, kind="ExternalInput").ap()
    wk = nc.dram_tensor("wk", [CCTX, FSH], F32, kind="ExternalInput").ap()
    wv = nc.dram_tensor("wv", [CCTX, FSH], F32, kind="ExternalInput").ap()
    wq = nc.dram_tensor("wq", [CIN, C], F32, kind="ExternalInput").ap()
    wo = nc.dram_tensor("wo", [C, C], F32, kind="ExternalInput").ap()
    biasin = nc.dram_tensor("biasin", [NKV, 1], F32, kind="ExternalInput").ap()
    onesin = nc.dram_tensor("onesin", [128, 64], F32, kind="ExternalInput").ap()
    eyein = nc.dram_tensor("eyein", [128, 128], F32, kind="ExternalInput").ap()
    yT = nc.dram_tensor("yT", [C, QTOK], F32, kind="ExternalOutput").ap()

    with tile.TileContext(nc) as tc:
        _build_graph(nc, tc, xT, ctxT, wk, wv, wq, wo, biasin, onesin, eyein, yT)
    nc.compile()
    return nc


def _build_graph(nc, tc, xT, ctxT, wk, wv, wq, wo, biasin, onesin, eyein, yT):
    Exp = mybir.ActivationFunctionType.Exp

    with (
        tc.tile_pool(name="dram", bufs=1, space="DRAM") as dram,
        tc.tile_pool(name="persist", bufs=1) as persist,
        tc.tile_pool(name="consts", bufs=1) as consts,
    ):
        # ---- constants
        ones_t = consts.tile([128, 64], BF16)
        nc.gpsimd.dma_start(ones_t[:], onesin)
        ones_sb = ones_t[:, 0:1]
        ones64_sb = ones_t[0:1, :]
        bias_sb = consts.tile([NKV, 1], F32)
        nc.sync.dma_start(bias_sb[:], biasin)
        eye_sb = consts.tile([128, 128], F32)
        nc.sync.dma_start(eye_sb[:], eyein)

        # ---- DRAM bounce buffers for the AllGather
        kT_ag_in = dram.tile([FSH, NKV], F32)
        v_ag_in = dram.tile([NKV, FSH], F32)
        kT_full = dram.tile([C, NKV], F32)
        v_full = dram.tile([4 * NKV, FSH], F32)

        # ================= Phase 1: k/v projection shards =================
        with (
            tc.tile_pool(name="p1sb", bufs=1) as p1sb,
            tc.tile_pool(name="p1ctx", bufs=6) as p1ctx,
            tc.tile_pool(name="p1w", bufs=6) as p1w,
            tc.tile_pool(name="p1psum", bufs=1, space="PSUM") as p1ps,
            tc.tile_pool(name="p1psum_t", bufs=2, space="PSUM") as p1ps_t,
        ):
            k_ps = p1ps.tile([NKV, FSH], F32)
            v_ps = p1ps.tile([NKV, FSH], F32)
            for c in range(CCH):
                ctx_t = p1ctx.tile([128, NKV], F32R, name="ctx_t")
                nc.sync.dma_start(ctx_t[:], ctxT[128 * c:128 * (c + 1), :].bitcast(F32R))
                wk_t = p1w.tile([128, FSH], F32R, name="wk_t")
                nc.sync.dma_start(wk_t[:], wk[128 * c:128 * (c + 1), :].bitcast(F32R))
                wv_t = p1w.tile([128, FSH], F32R, name="wv_t")
                nc.sync.dma_start(wv_t[:], wv[128 * c:128 * (c + 1), :].bitcast(F32R))
                nc.tensor.matmul(k_ps[:], ctx_t[:], wk_t[:],
                                 start=(c == 0), stop=(c == CCH - 1))
                nc.tensor.matmul(v_ps[:], ctx_t[:], wv_t[:],
                                 start=(c == 0), stop=(c == CCH - 1))

            # v shard: natural layout, straight to the AG input
            v_stage = p1sb.tile([NKV, FSH], F32)
            nc.scalar.copy(v_stage[:], v_ps[:])
            nc.sync.dma_start(v_ag_in[:], v_stage[:])

            # k shard: transpose (128kv, 384f) -> (384f, 128kv)
            k_nat = p1sb.tile([NKV, FSH], F32)
            nc.scalar.copy(k_nat[:], k_ps[:])
            kT_stage = p1sb.tile([128, 3 * NKV], F32)
            for s in range(3):
                kt_ps = p1ps_t.tile([128, 128], F32, name="kt_ps")
                nc.tensor.transpose(kt_ps[:], k_nat[:, 128 * s:128 * (s + 1)],
                                    eye_sb[:])
                nc.scalar.copy(kT_stage[:, 128 * s:128 * (s + 1)], kt_ps[:])
            nc.sync.dma_start(
                kT_ag_in.rearrange("(s p) k -> p s k", p=128),
                kT_stage.rearrange("p (s k) -> p s k", s=3))

        groups = [[0, 1, 2, 3], [4, 5, 6, 7]]
        nc.gpsimd.collective_compute(
            "AllGather", mybir.AluOpType.bypass, replica_groups=groups,
            ins=[kT_ag_in[:].opt()], outs=[kT_full[:].opt()])
        nc.gpsimd.collective_compute(
            "AllGather", mybir.AluOpType.bypass, replica_groups=groups,
            ins=[v_ag_in[:].opt()], outs=[v_full[:].opt()])

        kT_sb = persist.tile([128, C], F32R)
        nc.sync.dma_start(kT_sb.rearrange("p (t k) -> p t k", t=NCH),
                          kT_full.rearrange("(t p) k -> p t k", p=128).bitcast(F32R))
        v_sb = persist.tile([128, C], BF16)
        nc.gpsimd.dma_start(v_sb.rearrange("p (g f) -> p g f", g=4),
                            v_full.rearrange("(g p) f -> p g f", p=128))

        # ================= Phase 2a: q^T projection =================
        qT_sb = persist.tile([128, NCH * QTOK], F32R)
        with (
            tc.tile_pool(name="xtp", bufs=1) as xtp,
            tc.tile_pool(name="wqp", bufs=2) as wqp,
            tc.tile_pool(name="qtps", bufs=3, space="PSUM") as qtps,
        ):
            xT_sb = xtp.tile([128, NCH * QTOK], F32R)
            for c in range(NCH):
                nc.sync.dma_start(xT_sb[:, QTOK * c:QTOK * (c + 1)],
                                  xT[128 * c:128 * (c + 1), :].bitcast(F32R))
            for fc in range(NCH):
                wq_col = wqp.tile([128, C], F32R, name="wq_col")
                nc.sync.dma_start(
                    wq_col.rearrange("p (c f) -> p c f", c=NCH),
                    wq[:, 128 * fc:128 * (fc + 1)]
                    .rearrange("(c p) f -> p c f", p=128).bitcast(F32R))
                for qt in range(2):
                    q_ps = qtps.tile([128, 512], F32, name="q_ps")
                    for c in range(NCH):
                        nc.tensor.matmul(
                            q_ps[:],
                            wq_col[:, 128 * c:128 * (c + 1)],
                            xT_sb[:, QTOK * c + 512 * qt:QTOK * c + 512 * qt + 512],
                            start=(c == 0), stop=(c == NCH - 1))
                    nc.scalar.copy(
                        qT_sb[:, QTOK * fc + 512 * qt:QTOK * fc + 512 * qt + 512],
                        q_ps[:])

        # ================= Phase 2b: attention =================
        outT_sb = persist.tile([128, NCH * QTOK], F32R)
        with (
            tc.tile_pool(name="expp", bufs=4) as expp,
            tc.tile_pool(name="recipp", bufs=2) as recipp,
            tc.tile_pool(name="bcsb", bufs=2) as bcsb,
            tc.tile_pool(name="scps", bufs=3, space="PSUM") as scps,
            tc.tile_pool(name="denps", bufs=1, space="PSUM") as denps,
            tc.tile_pool(name="ovps", bufs=2, space="PSUM") as ovps,
            tc.tile_pool(name="bcps", bufs=1, space="PSUM") as bcps,
        ):
            for qt in range(2):
                for c2 in range(NCH):
                    den_ps = denps.tile([1, 1024], F32, name="den_ps")
                    ov_ps = ovps.tile([128, 512], F32, name="ov_ps")
                    exps = []
                    for hh in range(2):
                        h = 2 * c2 + hh
                        sc_ps = scps.tile([NKV, 512], F32, name="sc_ps")
                        nc.tensor.matmul(
                            sc_ps[:],
                            kT_sb[64 * hh:64 * hh + 64, 128 * c2:128 * (c2 + 1)],
                            qT_sb[64 * hh:64 * hh + 64,
                                  QTOK * c2 + 512 * qt:QTOK * c2 + 512 * qt + 512],
                            start=True, stop=True)
                        exp_sb = expp.tile([NKV, 512], BF16, name="exp_sb")
                        nc.scalar.activation(exp_sb[:], sc_ps[:], Exp,
                                             bias=bias_sb[:], scale=SCALE)
                        exps.append(exp_sb)
                        nc.tensor.matmul(den_ps[0:1, 512 * hh:512 * hh + 512],
                                         ones_sb,
                                         exp_sb[:], start=True, stop=True)
                    for hh in range(2):
                        h = 2 * c2 + hh
                        nc.tensor.matmul(
                            ov_ps[64 * hh:64 * hh + 64, :],
                            v_sb[:, 64 * h:64 * h + 64],
                            exps[hh][:], start=True, stop=True)
                    recip_sb = recipp.tile([1, 1024], BF16, name="recip_sb")
                    with nc.allow_low_precision(reason="bf16 softmax denominators"):
                        nc.vector.reciprocal(recip_sb[:], den_ps[:])
                    bc_ps = bcps.tile([128, 512], F32, name="bc_ps")
                    for hh in range(2):
                        nc.tensor.matmul(
                            bc_ps[64 * hh:64 * hh + 64, :], ones64_sb,
                            recip_sb[0:1, 512 * hh:512 * hh + 512],
                            start=True, stop=True)
                    bc_sb = bcsb.tile([128, 512], F32, name="bc_sb")
                    nc.scalar.copy(bc_sb[:], bc_ps[:])
                    nc.vector.tensor_mul(
                        outT_sb[:, QTOK * c2 + 512 * qt:QTOK * c2 + 512 * qt + 512],
                        ov_ps[:], bc_sb[:])

        # ================= Phase 2c: y^T projection =================
        with (
            tc.tile_pool(name="wop", bufs=2) as wop,
            tc.tile_pool(name="ytsb", bufs=3) as ytsb,
            tc.tile_pool(name="ytps", bufs=3, space="PSUM") as ytps,
        ):
            for oc in range(NCH):
                wo_col = wop.tile([128, C], F32R, name="wo_col")
                nc.sync.dma_start(
                    wo_col.rearrange("p (c f) -> p c f", c=NCH),
                    wo[:, 128 * oc:128 * (oc + 1)]
                    .rearrange("(c p) f -> p c f", p=128).bitcast(F32R))
                for qt in range(2):
                    y_ps = ytps.tile([128, 512], F32, name="y_ps")
                    for c in range(NCH):
                        nc.tensor.matmul(
                            y_ps[:],
                            wo_col[:, 128 * c:128 * (c + 1)],
                            outT_sb[:, QTOK * c + 512 * qt:QTOK * c + 512 * qt + 512],
                            start=(c == 0), stop=(c == NCH - 1))
                    y_sb = ytsb.tile([128, 512], F32, name="y_sb")
                    nc.scalar.copy(y_sb[:], y_ps[:])
                    nc.sync.dma_start(
                        yT[128 * oc:128 * (oc + 1), 512 * qt:512 * qt + 512],
                        y_sb[:])


_NC_CACHE = None


def _get_nc():
    global _NC_CACHE
    if _NC_CACHE is None:
        _NC_CACHE = build_nc()
    return _NC_CACHE


def kernel(x, context, context_mask, Wq, Wk, Wv, Wo):
    x = np.ascontiguousarray(np.asarray(x, dtype=np.float32))
    context = np.asarray(context, dtype=np.float32)
    context_mask = np.asarray(context_mask)
    Wq = np.ascontiguousarray(np.asarray(Wq, dtype=np.float32))
    Wk = np.asarray(Wk, dtype=np.float32)
    Wv = np.asarray(Wv, dtype=np.float32)
    Wo = np.ascontiguousarray(np.asarray(Wo, dtype=np.float32))

    eye = np.eye(128, dtype=np.float32)
    in_maps = []
    for i in range(NCORES):
        b, j = i // 4, i % 4
        bias = np.where(context_mask[b], 0.0, MASK_NEG).astype(np.float32)[:, None]
        in_maps.append({
            "xT": np.ascontiguousarray(x[b, QTOK * j:QTOK * (j + 1), :].T),
            "ctxT": np.ascontiguousarray(context[b].T),
            "wk": np.ascontiguousarray(Wk[:, FSH * j:FSH * (j + 1)]),
            "wv": np.ascontiguousarray(Wv[:, FSH * j:FSH * (j + 1)]),
            "wq": Wq,
            "wo": Wo,
            "biasin": bias,
            "onesin": np.ones((128, 64), dtype=np.float32),
            "eyein": eye,
        })

    nc = _get_nc()
    res = run_bass_kernel_spmd(nc, in_maps, core_ids=list(range(NCORES)))

    y = np.empty((B, NQ, C), dtype=np.float32)
    for i in range(NCORES):
        b, j = i // 4, i % 4
        y[b, QTOK * j:QTOK * (j + 1), :] = res.results[i]["yT"].T
    return y
